# revision 7
# baseline (speedup 1.0000x reference)
"""Luong 'general' attention scores + softmax on 8 Trainium2 NeuronCores.

Reference computes:
    energy = einsum('sbh,kh->sbk', enc, W) + b          # [S,B,H]
    scores = einsum('bh,sbh->bs', hidden[0], energy)    # [B,S]
    attn   = softmax(scores, axis=1)[:, None, :]        # [B,1,S]

Algebra: scores[b,s] = hidden[b] . (W @ enc[s,b]) + hidden[b] . bias.
The bias term is constant over s, so it cancels in the softmax.  With
q = hidden @ W  (tiny matmul), scores[b,s] = q[b] . enc[s,b].  The kernel
is HBM-bound on streaming enc exactly once — so enc travels as fp16
(empirical rel err vs the fp32 reference ~3e-3 against the 2e-2 gate),
halving the DMA floor from ~94 us to ~47 us per core.

Sharding: data-parallel over batch.  Core c gets batches [16c, 16c+16).
SBUF partitions pack p = b*8+g (batch-major); group g owns s in
[64g, 64g+64).  Batch-major packing makes the scores->softmax rearrange a
single DMA whose flat iteration orders match: scores[(b g), c] ->
scoresT[b, (g c)].  Host-permuted fp16 layouts make every DMA dense:
    enc16[b*8+g, sg*H+h] = fp16(enc[g*64+sg, 16*core+b, h])
    wh16[p, half*KC*512 + kc*512 + j] = fp16(W[kc*128+p, half*512+j])
    hid16[p, kc*128 + b*8+g] = fp16(hidden[0, 16*core+b, kc*128+p])

Measurement note (2026-08-10 session): an earlier session concluded the
DMA system capped at ~120-130 GB/s/core — that was an artifact of timing
through run_bass_kernel_spmd, which re-traces the program and re-uploads
all inputs every call.  With a persistent jitted callable +
device-resident inputs + pipelined dispatch (runner.py/bench.py), DMA-only
probes sustain ~350-480 GB/s/core on one HWDGE ring and ~560-900 GB/s/core
with chunks split across both HWDGE rings (sync+scalar), all 8 cores
streaming.  Under correct measurement this kernel runs ~37 us/iteration
and is DVE-bound (64 fused multiply+reduce columns x [128,1024] fp16 at
~245 G elem/s ~= 34 us), with the single-ring enc DMA (~40 us) right
behind.  Hardware-verified losses: GPSIMD Pool tensor_tensor columns
(KERNEL_NSTT=6/NTTD=0 + sp+act measured 94 us), so extra ring bandwidth
alone cannot help while DVE binds.  The next real win would be a hybrid
that offloads 2-3 of the 8 partition-groups' score columns to PE matmuls
(q as [K=128h,M=1] stationary, enc[h,s] moving, PSUM-accumulated over
h-chunks) + 2-ring DMA: est. ~28 us.
q = hidden @ W runs as fp16 matmuls (16x the fp32 PE rate) with the
group-broadcast lhsT trick producing qb[128, H] directly.  Each enc
chunk's 8 s-columns are split across engines (hardware-A/B'd): 4 fused
multiply+reduce STTs on DVE plus 4 DVE tensor_tensor multiplies whose
accumulating reduces run on ScalarE.  One strided DMA rearranges scores
into the [16, 512] softmax layout (batch-major partitions make the flat
orders match).  Softmax: reduce_max(negate) + exp(bias=-max, accum sum)
+ reciprocal + scale.
"""

import os
import sys

for _p in ("/opt/trn_rl_repo", "/root/.axon_site/_ro/trn_rl_repo"):
    if os.path.isdir(_p):
        sys.path.insert(0, _p)
        break

from contextlib import ExitStack

import numpy as np

import concourse.tile as tile
from concourse import bacc, mybir
from concourse.bass_utils import run_bass_kernel_spmd

S, B, H = 512, 128, 1024
NCORES = 8
BLOC = B // NCORES          # 16 batches per core
GROUPS = 8                  # partition groups; GROUPS * BLOC = 128
SG = S // GROUPS            # 64 s-values per group
KC = H // 128               # 8 contraction chunks for q = hidden @ W
CHUNK = 8                   # s-columns per enc DMA (2 MB per transfer)
NCHUNK = SG // CHUNK

FP32 = mybir.dt.float32
FP16 = mybir.dt.float16
MUL = mybir.AluOpType.mult

# Per-chunk column plan.  Real-ISA constraints: fused multiply+reduce
# (TensorScalarPtr) is DVE-only; Pool supports plain TensorTensor; every
# TT column's accumulating reduce runs on ACT.  The default (4 DVE fused
# + 4 DVE-TT/ACT-reduce, no Pool) measured fastest on hardware in
# interleaved A/B at 128-rep steady state; Pool TT (GPSIMD software
# loops) was a consistent loss there despite the cost model liking it.
N_STT = int(os.environ.get("KERNEL_NSTT", "8"))    # DVE fused cols
N_TTD = int(os.environ.get("KERNEL_NTTD", "0"))    # DVE TT + ACT reduce
# remaining cols: Pool TT + ACT reduce
SPLIT = [
    int(x)
    for x in os.environ.get(
        "KERNEL_SPLIT", ",".join(["8"] * (SG // CHUNK - 1)) + ",4,4"
    ).split(",")
]
assert sum(SPLIT) == SG
# Spread the enc stream over multiple DGE rings (SP HWDGE, ACT HWDGE,
# SWDGE): real hardware tops out well below the aggregate HBM bandwidth
# on any single ring.  "sp" = all on SP (what TimelineSim thinks is
# fine), "sp+sw" = alternate SP/SWDGE, "3way" = SP/ACT/SWDGE round-robin.
QSPLIT = os.environ.get("KERNEL_QSPLIT", "sp+act")
# CONTIG: host packs enc chunk-major so every chunk DMA reads one fully
# contiguous DRAM block (no 128 KB partition-stride jumps).
CONTIG = os.environ.get("KERNEL_CONTIG", "0") == "1"
# HIDBC: send hidT without the 8x group replication and replicate via a
# stride-0 broadcast dim in the matmul lhsT AP instead (saves 0.23 MB).
# (Rejected by neuronx-cc; kept for reference.)
HIDBC = os.environ.get("KERNEL_HIDBC", "0") == "1"
# WQ: which DGE ring carries the W halves — "sp" (in-line ahead of the
# enc stream) or "act" (side-stream on the ACT ring, concurrent with enc).
WQ = os.environ.get("KERNEL_WQ", "sp")
# DEADPROD: route the fused-STT columns' (unused) product outputs through
# a stride-0 broadcast AP onto one dead [128,1] tile instead of real
# 2 KB/partition tiles — only accum_out matters; trims SBUF write traffic.
DEADPROD = os.environ.get("KERNEL_DEADPROD", "0") == "1"

_cache = {}
LAST_RESULTS = None  # test harness reads exec_time_ns off this


def _build_nc(reps=1):
    """Build the kernel module.  reps>1 unrolls the whole kernel body
    back-to-back inside one program — used by the benchmark to measure the
    marginal per-iteration device time with the per-dispatch overhead of
    the host runtime amortized away.  kernel() always uses reps=1."""
    key = (
        "nc", N_STT, N_TTD, tuple(SPLIT), QSPLIT, CONTIG, HIDBC, WQ, DEADPROD,
        os.environ.get("KERNEL_EBUFS", "6"), reps,
    )
    if key in _cache:
        return _cache[key]

    # Bacc (not raw Bass): its compile pipeline legalizes sync waits to the
    # TRN2 1-wait-per-instruction limit and encodes InstISA subclasses.
    nc = bacc.Bacc(
        "TRN2",
        target_bir_lowering=False,
        debug=False,
        enable_asserts=True,
        num_devices=NCORES,
    )
    if CONTIG:
        enc_d = nc.dram_tensor(
            "enc", [1, 128 * SG * H], FP16, kind="ExternalInput"
        ).ap()
    else:
        enc_d = nc.dram_tensor("enc", [128, SG * H], FP16, kind="ExternalInput").ap()
    wh_d = nc.dram_tensor("wh", [128, KC * H], FP16, kind="ExternalInput").ap()
    hid_cols = KC * BLOC if HIDBC else KC * 128
    hid_d = nc.dram_tensor("hid", [128, hid_cols], FP16, kind="ExternalInput").ap()
    out = nc.dram_tensor("attn", [BLOC, S], FP32, kind="ExternalOutput").ap()

    with tile.TileContext(nc) as tc, ExitStack() as ctx:
        const_pool = ctx.enter_context(tc.tile_pool(name="const", bufs=1))
        w_pool = ctx.enter_context(tc.tile_pool(name="w", bufs=1))
        enc_pool = ctx.enter_context(
            tc.tile_pool(name="enc", bufs=int(os.environ.get("KERNEL_EBUFS", "6")))
        )
        scratch_pool = ctx.enter_context(tc.tile_pool(name="scratch", bufs=2))
        small_pool = ctx.enter_context(tc.tile_pool(name="small", bufs=1))
        psum_pool = ctx.enter_context(tc.tile_pool(name="psum", bufs=2, space="PSUM"))

        # ---- Phase 0: qb = broadcast(hidden @ W) straight out of PE ----
        # PE clock-gate warmup: a chain of dummy matmuls keeps the PE busy
        # through the W-load window so pe_busy_start ramps to the full
        # 2.4 GHz clock before the real q matmuls.
        wu = const_pool.tile([128, 512], FP16)
        nc.gpsimd.memset(wu[:], 1.0)
        wp = psum_pool.tile([1, 512], FP32, tag="wu")
        for _ in range(10):
            nc.tensor.matmul(wp[:], wu[:, 0:1], wu[:], start=True, stop=True)

        # Preload the Exp activation table off the critical tail.
        actwarm = const_pool.tile([16, 1], FP32)
        nc.scalar.activation(
            actwarm[:], wu[0:16, 0:1], mybir.ActivationFunctionType.Exp
        )

        for _rep in range(reps):
            _kernel_body(nc, tc, ctx, enc_d, wh_d, hid_d, out,
                         const_pool, w_pool, enc_pool, scratch_pool,
                         small_pool, psum_pool)

    nc.finalize()
    _cache[key] = nc
    return nc


def _kernel_body(nc, tc, ctx, enc_d, wh_d, hid_d, out, const_pool, w_pool,
                 enc_pool, scratch_pool, small_pool, psum_pool):
    if True:
        hid_sb = w_pool.tile([128, KC * BLOC if HIDBC else KC * 128], FP16,
                             tag="hid_sb")
        wh0_sb = w_pool.tile([128, KC * 512], FP16, tag="wh0_sb")
        wh1_sb = w_pool.tile([128, KC * 512], FP16, tag="wh1_sb")
        # Priority order on the serial DMA device: hid (ACT queue), then
        # W halves + enc chunks back-to-back on the SP queue.  Separate
        # half tiles so the half-0 matmuls unblock after the first piece.
        nc.scalar.dma_start(hid_sb[:], hid_d)
        half_w = KC * 512
        wq = nc.scalar if WQ == "act" else nc.sync
        wq.dma_start(wh0_sb[:], wh_d[:, :half_w])
        wq.dma_start(wh1_sb[:], wh_d[:, half_w:])

        qb = const_pool.tile([128, H], FP16, tag="qb")
        for half, wh_sb in enumerate((wh0_sb, wh1_sb)):
            qp = psum_pool.tile([128, 512], FP32, tag="qp")
            for kc in range(KC):
                if HIDBC:
                    # Group replication via a stride-0 broadcast dim: lhsT
                    # columns iterate (b, g) = output partition b*8+g.
                    lhsT = (
                        hid_sb[:, kc * BLOC : (kc + 1) * BLOC]
                        .unsqueeze(2)
                        .broadcast_to([128, BLOC, GROUPS])
                    )
                else:
                    lhsT = hid_sb[:, kc * 128 : (kc + 1) * 128]
                nc.tensor.matmul(
                    qp[:],
                    lhsT,
                    wh_sb[:, kc * 512 : (kc + 1) * 512],
                    start=(kc == 0),
                    stop=(kc == KC - 1),
                )
            nc.scalar.copy(qb[:, half * 512 : (half + 1) * 512], qp[:])

        # ---- Phase 1: stream enc; multiply+reduce split over 3 engines ----
        # scores[b*8+g, c] = q[b] . enc[g*64+c, b].
        scores = small_pool.tile([128, SG], FP32, tag="scores")
        scoresT = small_pool.tile([BLOC, S], FP32, tag="scoresT")
        # Chunk sizes: 8-col chunks for steady state, smaller tail chunks
        # so the end-of-stream compute trail shrinks.
        chunk_cols = SPLIT
        if QSPLIT == "sp":
            enc_q = [nc.sync]
        elif QSPLIT == "sp+sw":
            enc_q = [nc.sync, nc.gpsimd]
        elif QSPLIT == "sp+act":
            enc_q = [nc.sync, nc.scalar]
        else:
            enc_q = [nc.sync, nc.scalar, nc.gpsimd]
        col0 = 0
        for ch, ncols in enumerate(chunk_cols):
            et = enc_pool.tile([128, ncols * H], FP16, tag="enc")
            if CONTIG:
                # Chunk-major packing: the whole chunk is one contiguous
                # DRAM block; flat iteration orders match (dst is
                # partition-major over the same element order).
                off = 128 * col0 * H
                src = enc_d[:, off : off + 128 * ncols * H]
            else:
                src = enc_d[:, col0 * H : (col0 + ncols) * H]
            enc_q[ch % len(enc_q)].dma_start(et[:], src)
            # Column split across engines, scaled to the chunk width.  The
            # TT multiplies (DVE then Pool) are issued first so their ACT
            # reduces start early; DVE's fused STT columns follow.
            n_ttd = N_TTD * ncols // CHUNK
            n_stt = max(1, N_STT * ncols // CHUNK)
            n_stt = min(n_stt, ncols - n_ttd)
            plan = (
                [("ttd", j) for j in range(n_ttd)]
                + [("ttp", n_ttd + j) for j in range(ncols - n_stt - n_ttd)]
                + [("stt", ncols - n_stt + j) for j in range(n_stt)]
            )
            for kind, j in plan:
                col = col0 + j
                src = et[:, j * H : (j + 1) * H]
                sc = scores[:, col : col + 1]
                if kind == "stt":
                    if DEADPROD:
                        dead = scratch_pool.tile([128, 1], FP16, tag=f"dead{j}")
                        pout = dead[:].broadcast_to([128, H])
                    else:
                        prod = scratch_pool.tile([128, H], FP16, tag=f"prod{j}")
                        pout = prod[:]
                    nc.vector.scalar_tensor_tensor(
                        out=pout, in0=src, scalar=1.0, in1=qb[:],
                        op0=MUL, op1=MUL, accum_out=sc,
                    )
                    continue
                prod = scratch_pool.tile([128, H], FP16, tag=f"prod{j}")
                if kind == "ttd":
                    nc.vector.tensor_tensor(out=prod[:], in0=src, in1=qb[:], op=MUL)
                else:
                    nc.gpsimd.tensor_tensor(out=prod[:], in0=src, in1=qb[:], op=MUL)
                ascr = scratch_pool.tile([128, 1], FP32, tag=f"ascr{j}")
                nc.scalar.activation(
                    ascr[:].broadcast_to([128, H]),
                    prod[:],
                    mybir.ActivationFunctionType.Copy,
                    accum_out=sc,
                )
            col0 += ncols
        # Rearrange into softmax layout in ONE DMA: with batch-major
        # partitions the flat iteration orders match element-for-element:
        # scores[(b g), c] -> scoresT[b, (g c)].
        scoresT3 = scoresT[:].rearrange("b (g c) -> b g c", g=GROUPS)
        nc.sync.dma_start(scoresT3, scores[:])

        # ---- Phase 2: softmax over s per batch ----
        mx = small_pool.tile([BLOC, 1], FP32, tag="mx")
        nc.vector.reduce_max(mx[:], scoresT[:], axis=mybir.AxisListType.X, negate=True)
        probs = small_pool.tile([BLOC, S], FP32, tag="probs")
        ssum = small_pool.tile([BLOC, 1], FP32, tag="ssum")
        nc.scalar.activation(
            probs[:],
            scoresT[:],
            mybir.ActivationFunctionType.Exp,
            bias=mx[:],
            scale=1.0,
            accum_out=ssum[:],
        )
        # Normalize: reciprocal + scale, both on the DVE queue (no extra
        # cross-engine hop; HW tensor_scalar has no divide op).
        rsum = small_pool.tile([BLOC, 1], FP32, tag="rsum")
        nc.vector.reciprocal(rsum[:], ssum[:])
        attn_sb = small_pool.tile([BLOC, S], FP32, tag="attn_sb")
        nc.vector.tensor_scalar_mul(attn_sb[:], probs[:], rsum[:])
        nc.sync.dma_start(out, attn_sb[:])


def _prep_core_inputs(hid16_full, enc, w16, c):
    b0 = c * BLOC
    hid16 = np.ascontiguousarray(hid16_full[:, :, b0 : b0 + BLOC])  # [128, KC, 16]
    if HIDBC:
        hid16 = hid16.reshape(128, KC * BLOC)
    else:
        hid16 = np.repeat(hid16, GROUPS, axis=2).reshape(128, KC * 128)
    el = enc[:, b0 : b0 + BLOC, :]  # [512, 16, 1024]
    enc16 = np.ascontiguousarray(
        el.reshape(GROUPS, SG, BLOC, H)
        .transpose(2, 0, 1, 3)  # [b, g, sg, h] -> partitions p = b*8+g
        .reshape(128, SG * H)
        .astype(np.float16)
    )
    if CONTIG:
        pieces, col0 = [], 0
        for ncols in SPLIT:
            pieces.append(enc16[:, col0 * H : (col0 + ncols) * H].reshape(-1))
            col0 += ncols
        enc16 = np.concatenate(pieces).reshape(1, 128 * SG * H)
    return {"enc": enc16, "wh": w16, "hid": np.ascontiguousarray(hid16)}


def _prep_in_maps(inputs):
    hidden = np.asarray(inputs["hidden"], dtype=np.float32)
    enc = np.asarray(inputs["encoder_outputs"], dtype=np.float32)
    w = np.asarray(inputs["W_attn"], dtype=np.float32)
    # wh16[p, half*KC*512 + kc*512 + j] = W[kc*128+p, half*512+j]
    wr = w.reshape(KC, 128, 2, 512).transpose(1, 2, 0, 3).reshape(128, KC * H)
    w16 = np.ascontiguousarray(wr.astype(np.float16))
    # hid16_full[p, kc, b] = hidden[0, b, kc*128+p]
    hid16_full = np.ascontiguousarray(
        hidden[0].reshape(B, KC, 128).transpose(2, 1, 0).astype(np.float16)
    )
    return [_prep_core_inputs(hid16_full, enc, w16, c) for c in range(NCORES)]


def _warmup():
    """Compile + run once on dummy inputs at import time so the first real
    kernel() call hits the in-process XLA/NEFF caches instead of paying the
    multi-minute compile."""
    if _cache.get("warm") or os.environ.get("KERNEL_SKIP_WARMUP"):
        return
    try:
        kernel(
            np.zeros((1, B, H), np.float32),
            np.zeros((S, B, H), np.float32),
            np.zeros((H, H), np.float32),
            np.zeros((H,), np.float32),
        )
        _cache["warm"] = True
    except Exception:
        pass


def kernel(hidden, encoder_outputs, W_attn, b_attn=None, **_unused):
    global LAST_RESULTS
    nc = _build_nc()
    in_maps = _prep_in_maps(
        {"hidden": hidden, "encoder_outputs": encoder_outputs, "W_attn": W_attn}
    )
    res = run_bass_kernel_spmd(nc, in_maps, core_ids=list(range(NCORES)))
    LAST_RESULTS = res
    attn = np.concatenate([res.results[c]["attn"] for c in range(NCORES)], axis=0)
    return attn[:, None, :].astype(np.float32)


_warmup()



# revision 8
# speedup vs baseline: 2.5030x; 2.5030x over previous
"""Luong 'general' attention scores + softmax on 8 Trainium2 NeuronCores.

Reference computes:
    energy = einsum('sbh,kh->sbk', enc, W) + b          # [S,B,H]
    scores = einsum('bh,sbh->bs', hidden[0], energy)    # [B,S]
    attn   = softmax(scores, axis=1)[:, None, :]        # [B,1,S]

Algebra: scores[b,s] = hidden[b] . (W @ enc[s,b]) + hidden[b] . bias.
The bias term is constant over s, so it cancels in the softmax.  With
q = hidden @ W  (tiny matmul), scores[b,s] = q[b] . enc[s,b].  The kernel
is HBM-bound on streaming enc exactly once — so enc travels as fp16
(empirical rel err vs the fp32 reference ~3e-3 against the 2e-2 gate),
halving the DMA floor from ~94 us to ~47 us per core.

Sharding: data-parallel over batch.  Core c gets batches [16c, 16c+16).
SBUF partitions pack p = b*8+g (batch-major); group g owns s in
[64g, 64g+64).  Batch-major packing makes the scores->softmax rearrange a
single DMA whose flat iteration orders match: scores[(b g), c] ->
scoresT[b, (g c)].  Host-permuted fp16 layouts make every DMA dense:
    enc16[b*8+g, sg*H+h] = fp16(enc[g*64+sg, 16*core+b, h])
    wh16[p, half*KC*512 + kc*512 + j] = fp16(W[kc*128+p, half*512+j])
    hid16[p, kc*128 + b*8+g] = fp16(hidden[0, 16*core+b, kc*128+p])

Measurement note (2026-08-10 session): an earlier session concluded the
DMA system capped at ~120-130 GB/s/core — that was an artifact of timing
through run_bass_kernel_spmd, which re-traces the program and re-uploads
all inputs every call.  With a persistent jitted callable +
device-resident inputs + pipelined dispatch (runner.py/bench.py), DMA-only
probes sustain ~350-480 GB/s/core on one HWDGE ring and ~560-900 GB/s/core
with chunks split across both HWDGE rings (sync+scalar), all 8 cores
streaming.  Under correct measurement this kernel runs ~37 us/iteration
and is DVE-bound (64 fused multiply+reduce columns x [128,1024] fp16 at
~245 G elem/s ~= 34 us), with the single-ring enc DMA (~40 us) right
behind.  Hardware-verified losses: GPSIMD Pool tensor_tensor columns
(KERNEL_NSTT=6/NTTD=0 + sp+act measured 94 us), so extra ring bandwidth
alone cannot help while DVE binds.  The next real win would be a hybrid
that offloads 2-3 of the 8 partition-groups' score columns to PE matmuls
(q as [K=128h,M=1] stationary, enc[h,s] moving, PSUM-accumulated over
h-chunks) + 2-ring DMA: est. ~28 us.
q = hidden @ W runs as fp16 matmuls (16x the fp32 PE rate) with the
group-broadcast lhsT trick producing qb[128, H] directly.  Each enc
chunk's 8 s-columns are split across engines (hardware-A/B'd): 4 fused
multiply+reduce STTs on DVE plus 4 DVE tensor_tensor multiplies whose
accumulating reduces run on ScalarE.  One strided DMA rearranges scores
into the [16, 512] softmax layout (batch-major partitions make the flat
orders match).  Softmax: reduce_max(negate) + exp(bias=-max, accum sum)
+ reciprocal + scale.
"""

import os
import sys

for _p in ("/opt/trn_rl_repo", "/root/.axon_site/_ro/trn_rl_repo"):
    if os.path.isdir(_p):
        sys.path.insert(0, _p)
        break

from contextlib import ExitStack

import numpy as np

import concourse.tile as tile
from concourse import bacc, mybir
from concourse.bass_utils import run_bass_kernel_spmd

S, B, H = 512, 128, 1024
NCORES = 8
BLOC = B // NCORES          # 16 batches per core
GROUPS = 8                  # partition groups; GROUPS * BLOC = 128
SG = S // GROUPS            # 64 s-values per group
KC = H // 128               # 8 contraction chunks for q = hidden @ W
CHUNK = 8                   # s-columns per enc DMA (2 MB per transfer)
NCHUNK = SG // CHUNK

FP32 = mybir.dt.float32
FP16 = mybir.dt.float16
MUL = mybir.AluOpType.mult

# Per-chunk column plan.  Real-ISA constraints: fused multiply+reduce
# (TensorScalarPtr) is DVE-only; Pool supports plain TensorTensor; every
# TT column's accumulating reduce runs on ACT.  The default (4 DVE fused
# + 4 DVE-TT/ACT-reduce, no Pool) measured fastest on hardware in
# interleaved A/B at 128-rep steady state; Pool TT (GPSIMD software
# loops) was a consistent loss there despite the cost model liking it.
N_STT = int(os.environ.get("KERNEL_NSTT", "4"))    # DVE fused cols
N_TTD = int(os.environ.get("KERNEL_NTTD", "4"))    # DVE TT + ACT reduce
# remaining cols: Pool TT + ACT reduce
SPLIT = [
    int(x)
    for x in os.environ.get(
        "KERNEL_SPLIT", ",".join(["8"] * (SG // CHUNK - 1)) + ",4,4"
    ).split(",")
]
assert sum(SPLIT) == SG
# Spread the enc stream over multiple DGE rings (SP HWDGE, ACT HWDGE,
# SWDGE): real hardware tops out well below the aggregate HBM bandwidth
# on any single ring.  "sp" = all on SP (what TimelineSim thinks is
# fine), "sp+sw" = alternate SP/SWDGE, "3way" = SP/ACT/SWDGE round-robin.
QSPLIT = os.environ.get("KERNEL_QSPLIT", "sp")
# CONTIG: host packs enc chunk-major so every chunk DMA reads one fully
# contiguous DRAM block (no 128 KB partition-stride jumps).
CONTIG = os.environ.get("KERNEL_CONTIG", "0") == "1"
# HIDBC: send hidT without the 8x group replication and replicate via a
# stride-0 broadcast dim in the matmul lhsT AP instead (saves 0.23 MB).
# (Rejected by neuronx-cc; kept for reference.)
HIDBC = os.environ.get("KERNEL_HIDBC", "0") == "1"
# WQ: which DGE ring carries the W halves — "sp" (in-line ahead of the
# enc stream) or "act" (side-stream on the ACT ring, concurrent with enc).
WQ = os.environ.get("KERNEL_WQ", "sp")
# DEADPROD: route the fused-STT columns' (unused) product outputs through
# a stride-0 broadcast AP onto one dead [128,1] tile instead of real
# 2 KB/partition tiles — only accum_out matters; trims SBUF write traffic.
DEADPROD = os.environ.get("KERNEL_DEADPROD", "0") == "1"

_cache = {}
LAST_RESULTS = None  # test harness reads exec_time_ns off this


def _build_nc(reps=1):
    """Build the kernel module.  reps>1 unrolls the whole kernel body
    back-to-back inside one program — used by the benchmark to measure the
    marginal per-iteration device time with the per-dispatch overhead of
    the host runtime amortized away.  kernel() always uses reps=1."""
    key = (
        "nc", N_STT, N_TTD, tuple(SPLIT), QSPLIT, CONTIG, HIDBC, WQ, DEADPROD,
        os.environ.get("KERNEL_EBUFS", "6"), reps,
    )
    if key in _cache:
        return _cache[key]

    # Bacc (not raw Bass): its compile pipeline legalizes sync waits to the
    # TRN2 1-wait-per-instruction limit and encodes InstISA subclasses.
    nc = bacc.Bacc(
        "TRN2",
        target_bir_lowering=False,
        debug=False,
        enable_asserts=True,
        num_devices=NCORES,
    )
    if CONTIG:
        enc_d = nc.dram_tensor(
            "enc", [1, 128 * SG * H], FP16, kind="ExternalInput"
        ).ap()
    else:
        enc_d = nc.dram_tensor("enc", [128, SG * H], FP16, kind="ExternalInput").ap()
    wh_d = nc.dram_tensor("wh", [128, KC * H], FP16, kind="ExternalInput").ap()
    hid_cols = KC * BLOC if HIDBC else KC * 128
    hid_d = nc.dram_tensor("hid", [128, hid_cols], FP16, kind="ExternalInput").ap()
    out = nc.dram_tensor("attn", [BLOC, S], FP32, kind="ExternalOutput").ap()

    with tile.TileContext(nc) as tc, ExitStack() as ctx:
        const_pool = ctx.enter_context(tc.tile_pool(name="const", bufs=1))
        w_pool = ctx.enter_context(tc.tile_pool(name="w", bufs=1))
        enc_pool = ctx.enter_context(
            tc.tile_pool(name="enc", bufs=int(os.environ.get("KERNEL_EBUFS", "6")))
        )
        scratch_pool = ctx.enter_context(tc.tile_pool(name="scratch", bufs=2))
        small_pool = ctx.enter_context(tc.tile_pool(name="small", bufs=1))
        psum_pool = ctx.enter_context(tc.tile_pool(name="psum", bufs=2, space="PSUM"))

        # ---- Phase 0: qb = broadcast(hidden @ W) straight out of PE ----
        # PE clock-gate warmup: a chain of dummy matmuls keeps the PE busy
        # through the W-load window so pe_busy_start ramps to the full
        # 2.4 GHz clock before the real q matmuls.
        wu = const_pool.tile([128, 512], FP16)
        nc.gpsimd.memset(wu[:], 1.0)
        wp = psum_pool.tile([1, 512], FP32, tag="wu")
        for _ in range(10):
            nc.tensor.matmul(wp[:], wu[:, 0:1], wu[:], start=True, stop=True)

        # Preload the Exp activation table off the critical tail.
        actwarm = const_pool.tile([16, 1], FP32)
        nc.scalar.activation(
            actwarm[:], wu[0:16, 0:1], mybir.ActivationFunctionType.Exp
        )

        for _rep in range(reps):
            _kernel_body(nc, tc, ctx, enc_d, wh_d, hid_d, out,
                         const_pool, w_pool, enc_pool, scratch_pool,
                         small_pool, psum_pool)

    nc.finalize()
    _cache[key] = nc
    return nc


def _kernel_body(nc, tc, ctx, enc_d, wh_d, hid_d, out, const_pool, w_pool,
                 enc_pool, scratch_pool, small_pool, psum_pool):
    if True:
        hid_sb = w_pool.tile([128, KC * BLOC if HIDBC else KC * 128], FP16,
                             tag="hid_sb")
        wh0_sb = w_pool.tile([128, KC * 512], FP16, tag="wh0_sb")
        wh1_sb = w_pool.tile([128, KC * 512], FP16, tag="wh1_sb")
        # Priority order on the serial DMA device: hid (ACT queue), then
        # W halves + enc chunks back-to-back on the SP queue.  Separate
        # half tiles so the half-0 matmuls unblock after the first piece.
        nc.scalar.dma_start(hid_sb[:], hid_d)
        half_w = KC * 512
        wq = nc.scalar if WQ == "act" else nc.sync
        wq.dma_start(wh0_sb[:], wh_d[:, :half_w])
        wq.dma_start(wh1_sb[:], wh_d[:, half_w:])

        qb = const_pool.tile([128, H], FP16, tag="qb")
        for half, wh_sb in enumerate((wh0_sb, wh1_sb)):
            qp = psum_pool.tile([128, 512], FP32, tag="qp")
            for kc in range(KC):
                if HIDBC:
                    # Group replication via a stride-0 broadcast dim: lhsT
                    # columns iterate (b, g) = output partition b*8+g.
                    lhsT = (
                        hid_sb[:, kc * BLOC : (kc + 1) * BLOC]
                        .unsqueeze(2)
                        .broadcast_to([128, BLOC, GROUPS])
                    )
                else:
                    lhsT = hid_sb[:, kc * 128 : (kc + 1) * 128]
                nc.tensor.matmul(
                    qp[:],
                    lhsT,
                    wh_sb[:, kc * 512 : (kc + 1) * 512],
                    start=(kc == 0),
                    stop=(kc == KC - 1),
                )
            nc.scalar.copy(qb[:, half * 512 : (half + 1) * 512], qp[:])

        # ---- Phase 1: stream enc; multiply+reduce split over 3 engines ----
        # scores[b*8+g, c] = q[b] . enc[g*64+c, b].
        scores = small_pool.tile([128, SG], FP32, tag="scores")
        scoresT = small_pool.tile([BLOC, S], FP32, tag="scoresT")
        # Chunk sizes: 8-col chunks for steady state, smaller tail chunks
        # so the end-of-stream compute trail shrinks.
        chunk_cols = SPLIT
        if QSPLIT == "sp":
            enc_q = [nc.sync]
        elif QSPLIT == "sp+sw":
            enc_q = [nc.sync, nc.gpsimd]
        elif QSPLIT == "sp+act":
            enc_q = [nc.sync, nc.scalar]
        else:
            enc_q = [nc.sync, nc.scalar, nc.gpsimd]
        col0 = 0
        for ch, ncols in enumerate(chunk_cols):
            et = enc_pool.tile([128, ncols * H], FP16, tag="enc")
            if CONTIG:
                # Chunk-major packing: the whole chunk is one contiguous
                # DRAM block; flat iteration orders match (dst is
                # partition-major over the same element order).
                off = 128 * col0 * H
                src = enc_d[:, off : off + 128 * ncols * H]
            else:
                src = enc_d[:, col0 * H : (col0 + ncols) * H]
            enc_q[ch % len(enc_q)].dma_start(et[:], src)
            # Column split across engines, scaled to the chunk width.  The
            # TT multiplies (DVE then Pool) are issued first so their ACT
            # reduces start early; DVE's fused STT columns follow.
            n_ttd = N_TTD * ncols // CHUNK
            n_stt = max(1, N_STT * ncols // CHUNK)
            n_stt = min(n_stt, ncols - n_ttd)
            plan = (
                [("ttd", j) for j in range(n_ttd)]
                + [("ttp", n_ttd + j) for j in range(ncols - n_stt - n_ttd)]
                + [("stt", ncols - n_stt + j) for j in range(n_stt)]
            )
            for kind, j in plan:
                col = col0 + j
                src = et[:, j * H : (j + 1) * H]
                sc = scores[:, col : col + 1]
                if kind == "stt":
                    if DEADPROD:
                        dead = scratch_pool.tile([128, 1], FP16, tag=f"dead{j}")
                        pout = dead[:].broadcast_to([128, H])
                    else:
                        prod = scratch_pool.tile([128, H], FP16, tag=f"prod{j}")
                        pout = prod[:]
                    nc.vector.scalar_tensor_tensor(
                        out=pout, in0=src, scalar=1.0, in1=qb[:],
                        op0=MUL, op1=MUL, accum_out=sc,
                    )
                    continue
                prod = scratch_pool.tile([128, H], FP16, tag=f"prod{j}")
                if kind == "ttd":
                    nc.vector.tensor_tensor(out=prod[:], in0=src, in1=qb[:], op=MUL)
                else:
                    nc.gpsimd.tensor_tensor(out=prod[:], in0=src, in1=qb[:], op=MUL)
                ascr = scratch_pool.tile([128, 1], FP32, tag=f"ascr{j}")
                nc.scalar.activation(
                    ascr[:].broadcast_to([128, H]),
                    prod[:],
                    mybir.ActivationFunctionType.Copy,
                    accum_out=sc,
                )
            col0 += ncols
        # Rearrange into softmax layout in ONE DMA: with batch-major
        # partitions the flat iteration orders match element-for-element:
        # scores[(b g), c] -> scoresT[b, (g c)].
        scoresT3 = scoresT[:].rearrange("b (g c) -> b g c", g=GROUPS)
        nc.sync.dma_start(scoresT3, scores[:])

        # ---- Phase 2: softmax over s per batch ----
        mx = small_pool.tile([BLOC, 1], FP32, tag="mx")
        nc.vector.reduce_max(mx[:], scoresT[:], axis=mybir.AxisListType.X, negate=True)
        probs = small_pool.tile([BLOC, S], FP32, tag="probs")
        ssum = small_pool.tile([BLOC, 1], FP32, tag="ssum")
        nc.scalar.activation(
            probs[:],
            scoresT[:],
            mybir.ActivationFunctionType.Exp,
            bias=mx[:],
            scale=1.0,
            accum_out=ssum[:],
        )
        # Normalize: reciprocal + scale, both on the DVE queue (no extra
        # cross-engine hop; HW tensor_scalar has no divide op).
        rsum = small_pool.tile([BLOC, 1], FP32, tag="rsum")
        nc.vector.reciprocal(rsum[:], ssum[:])
        attn_sb = small_pool.tile([BLOC, S], FP32, tag="attn_sb")
        nc.vector.tensor_scalar_mul(attn_sb[:], probs[:], rsum[:])
        nc.sync.dma_start(out, attn_sb[:])


def _prep_core_inputs(hid16_full, enc, w16, c):
    b0 = c * BLOC
    hid16 = np.ascontiguousarray(hid16_full[:, :, b0 : b0 + BLOC])  # [128, KC, 16]
    if HIDBC:
        hid16 = hid16.reshape(128, KC * BLOC)
    else:
        hid16 = np.repeat(hid16, GROUPS, axis=2).reshape(128, KC * 128)
    el = enc[:, b0 : b0 + BLOC, :]  # [512, 16, 1024]
    enc16 = np.ascontiguousarray(
        el.reshape(GROUPS, SG, BLOC, H)
        .transpose(2, 0, 1, 3)  # [b, g, sg, h] -> partitions p = b*8+g
        .reshape(128, SG * H)
        .astype(np.float16)
    )
    if CONTIG:
        pieces, col0 = [], 0
        for ncols in SPLIT:
            pieces.append(enc16[:, col0 * H : (col0 + ncols) * H].reshape(-1))
            col0 += ncols
        enc16 = np.concatenate(pieces).reshape(1, 128 * SG * H)
    return {"enc": enc16, "wh": w16, "hid": np.ascontiguousarray(hid16)}


def _prep_in_maps(inputs):
    hidden = np.asarray(inputs["hidden"], dtype=np.float32)
    enc = np.asarray(inputs["encoder_outputs"], dtype=np.float32)
    w = np.asarray(inputs["W_attn"], dtype=np.float32)
    # wh16[p, half*KC*512 + kc*512 + j] = W[kc*128+p, half*512+j]
    wr = w.reshape(KC, 128, 2, 512).transpose(1, 2, 0, 3).reshape(128, KC * H)
    w16 = np.ascontiguousarray(wr.astype(np.float16))
    # hid16_full[p, kc, b] = hidden[0, b, kc*128+p]
    hid16_full = np.ascontiguousarray(
        hidden[0].reshape(B, KC, 128).transpose(2, 1, 0).astype(np.float16)
    )
    return [_prep_core_inputs(hid16_full, enc, w16, c) for c in range(NCORES)]


def _warmup():
    """Compile + run once on dummy inputs at import time so the first real
    kernel() call hits the in-process XLA/NEFF caches instead of paying the
    multi-minute compile."""
    if _cache.get("warm") or os.environ.get("KERNEL_SKIP_WARMUP"):
        return
    try:
        kernel(
            np.zeros((1, B, H), np.float32),
            np.zeros((S, B, H), np.float32),
            np.zeros((H, H), np.float32),
            np.zeros((H,), np.float32),
        )
        _cache["warm"] = True
    except Exception:
        pass


def kernel(hidden, encoder_outputs, W_attn, b_attn=None, **_unused):
    global LAST_RESULTS
    nc = _build_nc()
    in_maps = _prep_in_maps(
        {"hidden": hidden, "encoder_outputs": encoder_outputs, "W_attn": W_attn}
    )
    res = run_bass_kernel_spmd(nc, in_maps, core_ids=list(range(NCORES)))
    LAST_RESULTS = res
    attn = np.concatenate([res.results[c]["attn"] for c in range(NCORES)], axis=0)
    return attn[:, None, :].astype(np.float32)


_warmup()

